# revision 6
# baseline (speedup 1.0000x reference)
"""Trainium2 Bass kernel for the BOTE span-pool + biaffine model.

Contract: kernel(**inputs) takes the FULL unsharded inputs (numpy) and
returns the full outputs (ap_out [128,200,5], op_out [128,200,5],
triplet_out [128,200,200,4]) as a tuple, matching reference().

Strategy: pure data parallelism over the batch (128 -> 16 items on each of
8 NeuronCores), weights replicated.  On-device compute per core:
  - span mean-pooling as a mask matmul (mask built on device from
    `positions` with iota + scalar_tensor_tensor, 1/count pre-folded)
  - pos-tag embedding fused into the reduc matmul: one-hot(postag) matmul
    with W_pos = W_reduc[:, :50] @ embed_table.T (precomputed on device)
  - the four 400->150 FCs packed into one 400->600 matmul (rows permuted
    so every consumer slice is a clean partition chunk)
  - tag heads via a block-diagonal [300,10] weight
  - biaffine: affine = W_bi @ [ap_node;1], tri = affine @ op_node^T per item
All matmul operands fp16 in SBUF, accumulation fp32 in PSUM.
"""

import sys

if '/opt/trn_rl_repo' not in sys.path:
    sys.path.insert(0, '/opt/trn_rl_repo')

import numpy as np

import concourse.bass as bass
import concourse.tile as tile
from concourse import bacc, mybir
from concourse.bass_utils import run_bass_kernel_spmd

F16 = mybir.dt.float16
F32 = mybir.dt.float32
I32 = mybir.dt.int32

# problem shape (hardcoded per contract)
B, S, L, D = 128, 256, 200, 768
POS_VOCAB, POS_DIM = 60, 50
REDUC, HID, TAGS, POL = 400, 150, 5, 4
NCORES = 8
BI = B // NCORES            # items per core
NPAIR = BI // 2             # item pairs per core
R = BI * L                  # word rows per core (3200)
NW = 2 * L                  # moving-dim columns per pair (400)

# fc4 stacked-output permutation: physical chunks (128,128,128,128,88)
#   c0 = op_node h0:128, c1 = ap_rep h0:128, c2 = op_rep h0:128,
#   c3 = ap_node h0:128, c4 = [ap t22 | op t22 | apn t22 | opn t22]
_AP0, _OP0, _APN0, _OPN0 = 0, 150, 300, 450
PI = np.concatenate([
    _OPN0 + np.arange(128),
    _AP0 + np.arange(128),
    _OP0 + np.arange(128),
    _APN0 + np.arange(128),
    _AP0 + np.arange(128, 150),
    _OP0 + np.arange(128, 150),
    _APN0 + np.arange(128, 150),
    _OPN0 + np.arange(128, 150),
])
# affine output permutation: c0..c3 = pol p rows h0:128, c4 = 4x h128:150
SIG = np.concatenate(
    [p * HID + np.arange(128) for p in range(POL)]
    + [p * HID + np.arange(128, 150) for p in range(POL)]
)

_CACHE = {}


def _build_nc():
    nc = bacc.Bacc("TRN2", target_bir_lowering=False, debug=False)

    # ---- dram tensors -------------------------------------------------
    d_bert = nc.dram_tensor("bert", [BI, 2, 128, D], F16, kind="ExternalInput")
    d_pos0 = nc.dram_tensor("pos0", [R], I32, kind="ExternalInput")
    d_pos1 = nc.dram_tensor("pos1", [R], I32, kind="ExternalInput")
    d_ptag = nc.dram_tensor("ptag", [R], I32, kind="ExternalInput")
    d_WvT = nc.dram_tensor("WvT", [6, 128, REDUC], F16, kind="ExternalInput")
    d_WpT = nc.dram_tensor("WpT", [POS_DIM, REDUC], F16, kind="ExternalInput")
    d_embT = nc.dram_tensor("embT", [POS_DIM, POS_VOCAB], F16, kind="ExternalInput")
    d_breduc = nc.dram_tensor("breduc", [100, 4], F32, kind="ExternalInput")
    d_W4T = nc.dram_tensor("W4T", [4, 100, 600], F16, kind="ExternalInput")
    d_b4 = nc.dram_tensor("b4", [128, 5], F32, kind="ExternalInput")
    d_Wtag = nc.dram_tensor("Wtag", [300, 2 * TAGS], F16, kind="ExternalInput")
    d_btag = nc.dram_tensor("btag", [2 * TAGS, 1], F32, kind="ExternalInput")
    d_WbiT = nc.dram_tensor("WbiT", [HID, 600], F16, kind="ExternalInput")
    d_bbi = nc.dram_tensor("bbi", [128, 5], F32, kind="ExternalInput")

    d_tag = nc.dram_tensor("tagT", [2 * TAGS, R], F32, kind="ExternalOutput")
    d_trip = nc.dram_tensor("trip", [BI, L, L * POL], F32, kind="ExternalOutput")

    AF = mybir.ActivationFunctionType
    OP = mybir.AluOpType
    L2C = [(0, 128), (128, 72)]          # l2 chunks (offset, size)

    with tile.TileContext(nc) as tc:
        with (
            tc.tile_pool(name="const", bufs=1) as cpool,
            tc.tile_pool(name="pair", bufs=2) as ppool,
            tc.tile_pool(name="bert", bufs=3) as bpool,
            tc.tile_pool(name="mask", bufs=3) as mpool,
            tc.tile_pool(name="wv", bufs=2) as wvpool,
            tc.tile_pool(name="act", bufs=2) as apool,
            tc.tile_pool(name="trip", bufs=3) as tpool,
            tc.tile_pool(name="ps", bufs=3, space="PSUM") as pspool,
            tc.tile_pool(name="pstri", bufs=2, space="PSUM") as tripool,
        ):
            # ---- constants -------------------------------------------
            WvT = cpool.tile([128, 6, REDUC], F16)
            nc.sync.dma_start(out=WvT, in_=d_WvT.ap().transpose([1, 0, 2]))
            WpT = cpool.tile([POS_DIM, REDUC], F16)
            nc.sync.dma_start(out=WpT, in_=d_WpT.ap())
            embT = cpool.tile([POS_DIM, POS_VOCAB], F16)
            nc.sync.dma_start(out=embT, in_=d_embT.ap())
            W4T = cpool.tile([100, 4, 600], F16)
            nc.sync.dma_start(out=W4T, in_=d_W4T.ap().transpose([1, 0, 2]))
            Wtag = cpool.tile([128, 3, 2 * TAGS], F16)
            for c, (off, sz) in enumerate([(0, 128), (128, 128), (256, 44)]):
                nc.sync.dma_start(out=Wtag[:sz, c, :], in_=d_Wtag.ap()[off:off + sz, :])
            WbiT = cpool.tile([128, 2, 600], F16)
            nc.sync.dma_start(out=WbiT[:, 0, :], in_=d_WbiT.ap()[0:128, :])
            nc.sync.dma_start(out=WbiT[:22, 1, :], in_=d_WbiT.ap()[128:150, :])
            breduc = cpool.tile([100, 4], F32)
            nc.sync.dma_start(out=breduc, in_=d_breduc.ap())
            b4 = cpool.tile([128, 5], F32)
            nc.sync.dma_start(out=b4, in_=d_b4.ap())
            btag = cpool.tile([2 * TAGS, 1], F32)
            nc.sync.dma_start(out=btag, in_=d_btag.ap())
            bbi = cpool.tile([128, 5], F32)
            nc.sync.dma_start(out=bbi, in_=d_bbi.ap())

            iota = cpool.tile([128, 2], F32)
            nc.gpsimd.iota(iota, pattern=[[128, 2]], base=0,
                           channel_multiplier=1,
                           allow_small_or_imprecise_dtypes=True)

            # W_pos = (W_reduc[:, :50] @ embed_table.T).T : [60, 400]
            ps_wp = pspool.tile([POS_VOCAB, REDUC], F32, tag="mm")
            nc.tensor.matmul(ps_wp, embT, WpT, start=True, stop=True)
            WposT = cpool.tile([POS_VOCAB, REDUC], F16)
            nc.scalar.copy(WposT, ps_wp)

            for q in range(NPAIR):
                qoff = q * NW
                # ---- per-pair broadcast rows + one-hot ----------------
                p0rep = ppool.tile([128, NW], I32)
                nc.sync.dma_start(
                    out=p0rep, in_=bass.AP(d_pos0, qoff, [[0, 128], [1, NW]]))
                p1rep = ppool.tile([128, NW], I32)
                nc.sync.dma_start(
                    out=p1rep, in_=bass.AP(d_pos1, qoff, [[0, 128], [1, NW]]))
                ptagrep = ppool.tile([POS_VOCAB, NW], I32)
                nc.sync.dma_start(
                    out=ptagrep, in_=bass.AP(d_ptag, qoff, [[0, POS_VOCAB], [1, NW]]))
                cnt = ppool.tile([128, NW], F32)
                nc.vector.scalar_tensor_tensor(
                    out=cnt, in0=p1rep, scalar=1, in1=p0rep,
                    op0=OP.add, op1=OP.subtract)
                rec = ppool.tile([128, NW], F32)
                nc.vector.reciprocal(rec, cnt)
                onehot = ppool.tile([POS_VOCAB, NW], F16)
                nc.vector.tensor_scalar(
                    out=onehot, in0=ptagrep, scalar1=iota[:POS_VOCAB, 0:1],
                    scalar2=None, op0=OP.is_equal)

                # ---- per-item masks + bert + pooling ------------------
                wv = wvpool.tile([128, 6, 2, L], F16)
                for ii in range(2):
                    i = 2 * q + ii
                    ioff = ii * L
                    bert = bpool.tile([128, 2, D], F16)
                    nc.sync.dma_start(
                        out=bert, in_=d_bert.ap()[i].transpose([1, 0, 2]))
                    msk = mpool.tile([128, 2, L], F16)
                    tmp = mpool.tile([128, 2, L], F32)
                    for sc in range(2):
                        nc.vector.scalar_tensor_tensor(
                            out=tmp[:, sc, :], in0=p0rep[:, ioff:ioff + L],
                            scalar=iota[:, sc:sc + 1], in1=rec[:, ioff:ioff + L],
                            op0=OP.is_le, op1=OP.mult)
                        nc.vector.scalar_tensor_tensor(
                            out=msk[:, sc, :], in0=p1rep[:, ioff:ioff + L],
                            scalar=iota[:, sc:sc + 1], in1=tmp[:, sc, :],
                            op0=OP.is_ge, op1=OP.mult)
                    for dc in range(6):
                        ps = pspool.tile([128, L], F32, tag="mm")
                        for sc in range(2):
                            nc.tensor.matmul(
                                ps, bert[:, sc, dc * 128:(dc + 1) * 128],
                                msk[:, sc, :], start=(sc == 0), stop=(sc == 1))
                        nc.scalar.copy(wv[:, dc, ii, :], ps)

                # ---- reduc: [400, NW] = W_vT.T @ wv + WposT.T @ onehot
                reducT = apool.tile([100, 4, NW], F16)
                for mc in range(4):
                    ps = pspool.tile([100, NW], F32, tag="mm")
                    for kc in range(6):
                        nc.tensor.matmul(
                            ps, WvT[:, kc, mc * 100:(mc + 1) * 100],
                            wv[:, kc, :, :], start=(kc == 0), stop=False)
                    nc.tensor.matmul(
                        ps, WposT[:, mc * 100:(mc + 1) * 100], onehot,
                        start=False, stop=True)
                    nc.scalar.activation(
                        reducT[:, mc, :], ps, AF.Identity,
                        bias=breduc[:, mc:mc + 1], scale=1.0)

                # ---- fc4: [600pi, NW], relu ---------------------------
                fc4T = apool.tile([128, 5, NW], F16)
                for mc, (moff, msz) in enumerate(
                        [(0, 128), (128, 128), (256, 128), (384, 128), (512, 88)]):
                    ps = pspool.tile([128, NW], F32, tag="mm")
                    for kc in range(4):
                        nc.tensor.matmul(
                            ps[:msz, :], W4T[:, kc, moff:moff + msz],
                            reducT[:, kc, :], start=(kc == 0), stop=(kc == 3))
                    nc.scalar.activation(
                        fc4T[:msz, mc, :], ps[:msz, :], AF.Relu,
                        bias=b4[:msz, mc:mc + 1], scale=1.0)
                # tail groups relocated to base-0 tiles (matmul operands
                # must start at partition 0/32/64 with equal bases)
                apn_t = ppool.tile([22, NW], F16)
                nc.sync.dma_start(out=apn_t, in_=fc4T[44:66, 4, :])
                opn_t = ppool.tile([22, NW], F16)
                nc.sync.dma_start(out=opn_t, in_=fc4T[66:88, 4, :])

                # ---- tag heads: [10, NW] ------------------------------
                ps = pspool.tile([2 * TAGS, NW], F32, tag="mm")
                nc.tensor.matmul(ps, Wtag[:, 0, :], fc4T[:, 1, :],
                                 start=True, stop=False)
                nc.tensor.matmul(ps, Wtag[:, 1, :], fc4T[:, 2, :],
                                 start=False, stop=False)
                nc.tensor.matmul(ps, Wtag[:44, 2, :], fc4T[:44, 4, :],
                                 start=False, stop=True)
                tago = ppool.tile([2 * TAGS, NW], F32)
                nc.scalar.activation(tago, ps, AF.Identity, bias=btag, scale=1.0)
                nc.sync.dma_start(out=d_tag.ap()[:, qoff:qoff + NW], in_=tago)

                # ---- affine: [600sig, NW] = WbiT.T @ ap_node + b ------
                affT = apool.tile([128, 5, NW], F16)
                for mc in range(5):
                    moff = mc * 128
                    msz = 128 if mc < 4 else 88
                    ps = pspool.tile([128, NW], F32, tag="mm")
                    nc.tensor.matmul(
                        ps[:msz, :], WbiT[:, 0, moff:moff + msz],
                        fc4T[:, 3, :], start=True, stop=False)
                    nc.tensor.matmul(
                        ps[:msz, :], WbiT[:22, 1, moff:moff + msz],
                        apn_t, start=False, stop=True)
                    nc.scalar.activation(
                        affT[:msz, mc, :], ps[:msz, :], AF.Identity,
                        bias=bbi[:msz, mc:mc + 1], scale=1.0)
                aff_t = ppool.tile([22, 4, NW], F16)
                for p in range(POL):
                    nc.sync.dma_start(out=aff_t[:, p, :],
                                      in_=affT[p * 22:(p + 1) * 22, 4, :])

                # ---- tri per item: [l2, l1, pol] ----------------------
                for ii in range(2):
                    i = 2 * q + ii
                    ioff = ii * L
                    for l2c, (l2off, l2sz) in enumerate(L2C):
                        tsb = tpool.tile([128, L, POL], F32)
                        for pg in range(2):
                            pst = tripool.tile([128, 2, 512], F32, tag="tri")
                            for kc in range(2):
                                if kc == 0:
                                    lhs = fc4T[:, 0,
                                               ioff + l2off:ioff + l2off + l2sz]
                                else:
                                    lhs = opn_t[:, ioff + l2off:ioff + l2off + l2sz]
                                for pp in range(2):
                                    p = 2 * pg + pp
                                    if kc == 0:
                                        rhs = affT[:, p, ioff:ioff + L]
                                    else:
                                        rhs = aff_t[:, p, ioff:ioff + L]
                                    nc.tensor.matmul(
                                        pst[:l2sz, pp, 0:L], lhs, rhs,
                                        start=(kc == 0), stop=(kc == 1))
                            nc.vector.tensor_copy(
                                tsb[:l2sz, :, 2 * pg:2 * pg + 2].transpose([0, 2, 1]),
                                pst[:l2sz, :, 0:L])
                        nc.sync.dma_start(
                            out=d_trip.ap()[i, l2off:l2off + l2sz, :],
                            in_=tsb[:l2sz, :, :].rearrange("p l f -> p (l f)"))

    nc.compile()
    return nc


def _prep_weights(inputs):
    f16 = np.float16
    W_reduc = inputs["W_reduc"].astype(np.float32)
    WvT = np.ascontiguousarray(
        W_reduc[:, POS_DIM:].T.reshape(6, 128, REDUC)).astype(f16)
    WpT = np.ascontiguousarray(W_reduc[:, :POS_DIM].T).astype(f16)
    embT = np.ascontiguousarray(inputs["embed_table"].T).astype(f16)
    breduc = np.ascontiguousarray(
        inputs["b_reduc"].astype(np.float32).reshape(4, 100).T)

    W4 = np.vstack([inputs["W_ap"], inputs["W_op"],
                    inputs["W_ap2"], inputs["W_op2"]])[PI]
    W4T = np.ascontiguousarray(W4.T.reshape(REDUC, 600)).astype(np.float32)
    W4T = np.ascontiguousarray(W4T.reshape(4, 100, 600)).astype(f16)
    b4v = np.concatenate([inputs["b_ap"], inputs["b_op"],
                          inputs["b_ap2"], inputs["b_op2"]])[PI]
    b4 = np.zeros((128, 5), np.float32)
    for c in range(5):
        sz = 128 if c < 4 else 88
        b4[:sz, c] = b4v[c * 128:c * 128 + sz]

    # block tag weight rows match fc4 physical chunks c1, c2, c4[:44]
    Wtag = np.zeros((300, 2 * TAGS), np.float32)
    Wtag[0:128, 0:TAGS] = inputs["W_aptag"].T[0:128]
    Wtag[128:256, TAGS:] = inputs["W_optag"].T[0:128]
    Wtag[256:278, 0:TAGS] = inputs["W_aptag"].T[128:150]
    Wtag[278:300, TAGS:] = inputs["W_optag"].T[128:150]
    btag = np.concatenate([inputs["b_aptag"], inputs["b_optag"]])[:, None]
    btag = btag.astype(np.float32)

    Wbi_p = inputs["W_bi"][SIG]
    WbiT = np.ascontiguousarray(Wbi_p[:, :HID].T).astype(f16)
    bbiv = Wbi_p[:, HID].astype(np.float32)
    bbi = np.zeros((128, 5), np.float32)
    for c in range(5):
        sz = 128 if c < 4 else 88
        bbi[:sz, c] = bbiv[c * 128:c * 128 + sz]

    return dict(WvT=WvT, WpT=WpT, embT=embT, breduc=breduc.copy(),
                W4T=W4T, b4=b4, Wtag=Wtag.astype(f16), btag=btag,
                WbiT=WbiT, bbi=bbi)


def kernel(**inputs):
    if "nc" not in _CACHE:
        _CACHE["nc"] = _build_nc()
    nc = _CACHE["nc"]

    wmap = _prep_weights(inputs)
    bert = inputs["bert_vectors"].astype(np.float16)
    bert = bert.reshape(NCORES, BI, 2, 128, D)
    pos = inputs["positions"].reshape(NCORES, R, 2)
    ptag = np.ascontiguousarray(inputs["postag"].reshape(NCORES, R))

    in_maps = []
    for c in range(NCORES):
        m = dict(wmap)
        m["bert"] = np.ascontiguousarray(bert[c])
        m["pos0"] = np.ascontiguousarray(pos[c, :, 0])
        m["pos1"] = np.ascontiguousarray(pos[c, :, 1])
        m["ptag"] = ptag[c]
        in_maps.append(m)

    res = run_bass_kernel_spmd(nc, in_maps, core_ids=list(range(NCORES)))
    _CACHE["last_result"] = res

    ap_out = np.empty((B, L, TAGS), np.float32)
    op_out = np.empty((B, L, TAGS), np.float32)
    trip = np.empty((B, L, L, POL), np.float32)
    for c in range(NCORES):
        r = res.results[c]
        t = r["tagT"].reshape(2 * TAGS, BI, L)
        ap_out[c * BI:(c + 1) * BI] = t[:TAGS].transpose(1, 2, 0)
        op_out[c * BI:(c + 1) * BI] = t[TAGS:].transpose(1, 2, 0)
        trip[c * BI:(c + 1) * BI] = r["trip"].reshape(BI, L, L, POL)
    return ap_out, op_out, trip
